# revision 1
# baseline (speedup 1.0000x reference)
"""Trainium2 Bass kernel (raw Bass, explicit semaphores) for a BiDAF-style
attention-flow layer.

Math (per batch b):
    S[t,j] = c.w_c + q.w_q + (c*q).w_cq, masked by (t<con_len)&(j<qu_len)
    c2q    = softmax_j(S) @ Q
    value  = softmax_t(max_j S);  q2c = sum_t value[t] * C[t]
    G      = [C, c2q, C*c2q, C*q2c] * t_valid

Sharding: data-parallel over batch B=32 across 8 NeuronCores (4 each).
Device notes:
  - row-constant terms (c_proj, t-mask) cancel in the softmax over j; the
    value path uses exp(max_j S) = max_j exp(S) so no extra max pass.
  - no max-subtraction (randn scores are O(10); masked -> exp(-1e30)=0).
  - context rows with t >= con_len are pre-zeroed on host, so the G0
    block is a plain copy and all zeroing flows through products.
  - two-pass emission: dry pass records semaphore values, real pass
    emits standalone wait_ge commands (HW allows only ~1 attached wait
    per compute instruction, so waits must be discrete).
  - quirks honored: gpsimd is out-of-order (per-op/per-slot sems);
    matmul PSUM outputs must start at partition 0/32/64; M=1 matmuls
    cannot accumulate (start=False) -> q2c computed transposed M=128;
    TensorTensor reads at most one PSUM operand; no divide ALU op.
"""

import sys
import functools

for _p in ("/opt/trn_rl_repo",):
    if _p not in sys.path:
        sys.path.insert(0, _p)

import numpy as np
import concourse.bass as bass
from concourse import mybir

T, J, B, D = 1024, 128, 32, 256
NCORES = 8
BL = B // NCORES
NT = T // 128
NCT = BL * NT  # 32 chunks
NG = 16
NEG = -1.0e30

DMA_SEMS = set(["ws", "q0", "q1", "c0", "c1", "m0", "m1"] + [f"g{i}" for i in range(NG)])
F32 = mybir.dt.float32
AX = mybir.AxisListType.X
EXP = mybir.ActivationFunctionType.Exp
DIV = mybir.AluOpType.divide
ADD = mybir.AluOpType.add


class Em:
    """Per-engine emitter: dry pass counts sem values, real pass emits."""

    def __init__(self, dry, ctr, ev, eng=None, sems=None, own=None):
        self.dry = dry
        self.ctr = ctr
        self.ev = ev
        self.eng = eng
        self.sems = sems
        self.own = own
        self.waited = {}

    def do(self, fn, sem=None, tag=None):
        inst = None if self.dry else fn()
        if sem is not None:
            step = 16 if sem in DMA_SEMS else 1
            if inst is not None:
                inst.then_inc(self.sems[sem], step)
            self.ctr[sem] += step
            if tag is not None:
                self.ev[tag] = (sem, self.ctr[sem])
        return inst

    def mark(self, tag, sem):
        self.ev[tag] = (sem, self.ctr[sem])

    def w(self, tag):
        if self.dry:
            return
        if tag not in self.ev:
            return
        sem, val = self.ev[tag]
        if val <= 0:
            return
        if self.waited.get(sem, 0) >= val:
            return
        self.eng.wait_ge(self.sems[sem], val)
        self.waited[sem] = val


def build():
    nc = bass.Bass("TRN2", target_bir_lowering=False, debug=False)

    ctx_d = nc.dram_tensor("context", (T, BL, D), F32, kind="ExternalInput").ap()
    q_d = nc.dram_tensor("question", (J, BL, D), F32, kind="ExternalInput").ap()
    ws_d = nc.dram_tensor("wsT", (128, 6), F32, kind="ExternalInput").ap()
    t01_d = nc.dram_tensor("t01t", (BL, 128, NT), F32, kind="ExternalInput").ap()
    tm_d = nc.dram_tensor("tmaskt", (BL, 128, NT), F32, kind="ExternalInput").ap()
    jm_d = nc.dram_tensor("jmq", (BL, 1, J), F32, kind="ExternalInput").ap()
    out_d = nc.dram_tensor("out", (BL, T, 4 * D), F32, kind="ExternalOutput").ap()

    A = lambda name, shape: nc.alloc_sbuf_tensor(name, list(shape), F32).ap()
    P = lambda name, shape: nc.alloc_psum_tensor(name, list(shape), F32).ap()

    ident = A("ident", (128, 128))
    ones_row = A("ones_row", (1, 128))
    ones_col = A("ones_col", (128, 1))
    ws = A("ws", (128, 6))
    qn = [A(f"qn{i}", (128, D)) for i in range(2)]
    qt = [A(f"qt{i}", (128, 256)) for i in range(2)]
    qwt = [A(f"qwt{i}", (128, 256)) for i in range(2)]
    qpj = [A(f"qpj{i}", (1, J)) for i in range(2)]
    jmq = [A(f"jmq{i}", (1, J)) for i in range(2)]
    t018 = [A(f"t018_{i}", (128, NT)) for i in range(2)]
    tm8 = [A(f"tm8_{i}", (128, NT)) for i in range(2)]
    cna = [A(f"cna{i}", (128, NT, D)) for i in range(2)]
    ctc = [A(f"ctc{i}", (128, 256)) for i in range(4)]
    p_t = [A(f"p{i}", (128, 128)) for i in range(4)]
    pts = [A(f"pts{i}", (128, 128)) for i in range(4)]
    ssum = [A(f"ssum{i}", (128, 1)) for i in range(4)]
    rs01 = [A(f"rs01_{i}", (128, 1)) for i in range(4)]
    rcp = [A(f"rcp_{i}", (128, 1)) for i in range(4)]
    pm8 = [A(f"pm8_{i}", (128, NT)) for i in range(2)]
    x1 = [A(f"x1_{i}", (128, NT)) for i in range(2)]
    ex8 = [A(f"ex8_{i}", (128, NT)) for i in range(2)]
    e8 = [A(f"e8_{i}", (128, NT)) for i in range(2)]
    sums8 = [A(f"sums8_{i}", (NT, 1)) for i in range(2)]
    rtot = [A(f"rtot_{i}", (1, 1)) for i in range(2)]
    q2c_sb = [A(f"q2c_sb{i}", (1, D)) for i in range(2)]
    q2cTs = [A(f"q2cTs{i}", (128, 2)) for i in range(2)]
    q2cb = [A(f"q2cb{i}", (128, D)) for i in range(2)]
    g = [A(f"g{i}", (128, 4 * D)) for i in range(NG)]

    sful = [P(f"sful{i}", (128, 512)) for i in range(3)]  # [S | CT-pair]
    trp = [P(f"trp{i}", (128, 512)) for i in range(2)]  # PT / (qt-pair hi half)
    c2qp = [P(f"c2qp{i}", (128, 512)) for i in range(2)]  # c2q lo, q2cb hi
    auxp = P("auxp", (128, 512))
    # aux bank layout (all disjoint):
    cp8 = auxp[:, 0:NT]
    q2cT = [auxp[:, 8:9], auxp[:, 9:10]]  # q2c^T halves (d on partitions)
    sums8_ps = auxp[0:NT, 10:11]
    tot_ps = auxp[0:1, 12:13]
    q2c_row = auxp[0:1, 16 : 16 + D]  # transposed back to a row
    qp_ps = [trp[1][0:1, 256:384], trp[1][0:1, 384:512]]  # q_proj halves

    sem_names = (["ws", "q0", "q1", "c0", "c1", "m0", "m1", "pe", "act", "dve", "pool"]
                 + [f"g{i}" for i in range(NG)] + [f"p{i}" for i in range(NG)])
    sems = {n: nc.alloc_semaphore(f"sem_{n}") for n in sem_names}

    # ------------------------------------------------------------------ streams
    def stream_sync(X):
        X.do(lambda: nc.sync.dma_start(out=ws, in_=ws_d), "ws", "ws")

        def stores_for(b):
            for h in range(NT):
                k = b * NT + h
                X.w(f"G2_{k}"); X.w(f"G1_{k}"); X.w(f"G0_{k}")
                X.do(lambda h=h, k=k: nc.sync.dma_start(
                    out=out_d[b, h * 128 : (h + 1) * 128, 0:768],
                    in_=g[k % NG][:, 0:768]), f"g{k % NG}", f"store_a{k}")
            for h in range(NT):
                k = b * NT + h
                X.w(f"G3_{k}")
                X.do(lambda h=h, k=k: nc.sync.dma_start(
                    out=out_d[b, h * 128 : (h + 1) * 128, 768:1024],
                    in_=g[k % NG][:, 768:1024]), f"g{k % NG}", f"gfree_{k}")

        for b in range(BL):
            be = b % 2
            X.w(f"qn_free{b-2}")
            X.do(lambda b=b, be=be: nc.sync.dma_start(out=qn[be], in_=q_d[:, b, :]),
                 f"q{be}", f"qn{b}")
            X.w(f"cna_free{b-2}")
            X.do(lambda b=b, be=be: nc.sync.dma_start(
                out=cna[be], in_=ctx_d[:, b, :].rearrange("(c p) d -> p c d", p=128)),
                f"c{be}", f"cna{b}")
            X.w(f"masks_free{b-2}")
            X.do(lambda b=b, be=be: nc.sync.dma_start(out=t018[be], in_=t01_d[b]), f"m{be}")
            X.do(lambda b=b, be=be: nc.sync.dma_start(out=tm8[be], in_=tm_d[b]), f"m{be}")
            X.do(lambda b=b, be=be: nc.sync.dma_start(out=jmq[be], in_=jm_d[b]),
                 f"m{be}", f"masks{b}")
            if b >= 1:
                stores_for(b - 1)
        stores_for(BL - 1)

    def stream_pool(X):
        NE = mybir.AluOpType.not_equal
        X.do(lambda: nc.gpsimd.memset(ident, 0.0), "pool", "identms")
        if not X.dry:
            X.eng.wait_ge(sems["pool"], X.ev["identms"][1])
        X.do(lambda: nc.gpsimd.affine_select(
            out=ident, in_=ident, compare_op=NE, fill=1.0, base=0,
            pattern=[[-1, 128]], channel_multiplier=1), "pool")
        X.do(lambda: nc.gpsimd.memset(ones_row, 1.0), "pool")
        X.do(lambda: nc.gpsimd.memset(ones_col, 1.0), "pool", "consts")
        for b in range(BL):
            be = b % 2
            X.w(f"cna{b}")
            for h in range(NT):
                k = b * NT + h
                X.w(f"gfree_{k - NG}")
                X.do(lambda k=k, h=h, be=be: nc.gpsimd.tensor_copy(
                    g[k % NG][:, 0:256], cna[be][:, h, :]), f"p{k % NG}", f"G0_{k}")
                kc = k - 3
                if kc >= b * NT:
                    X.w(f"G1_{kc}")
                    X.w(f"G0_{kc}")
                    X.do(lambda kc=kc: nc.gpsimd.tensor_mul(
                        g[kc % NG][:, 512:768], g[kc % NG][:, 0:256], g[kc % NG][:, 256:512]),
                        f"p{kc % NG}", f"G2_{kc}")
            for kc in (b * NT + NT - 3, b * NT + NT - 2, b * NT + NT - 1):
                X.w(f"G1_{kc}")
                X.w(f"G0_{kc}")
                X.do(lambda kc=kc: nc.gpsimd.tensor_mul(
                    g[kc % NG][:, 512:768], g[kc % NG][:, 0:256], g[kc % NG][:, 256:512]),
                    f"p{kc % NG}", f"G2_{kc}")
            X.w(f"q2cbcopy{b}")
            for h in range(NT):
                k = b * NT + h
                X.do(lambda k=k, be=be: nc.gpsimd.tensor_mul(
                    g[k % NG][:, 768:1024], g[k % NG][:, 0:256], q2cb[be]),
                    f"p{k % NG}", f"G3_{k}")


    def stream_pe(X):
        X.w("consts")  # ident ready (consts is last gpsimd init op)
        X.w("ws")
        for b in range(BL):
            be = b % 2
            # question transposes into trp[0] hi half
            X.w(f"qn{b}")
            X.w(f"qtcopy{b-1}")  # trp[0][:,256:512] free
            X.do(lambda be=be: nc.tensor.transpose(trp[0][:, 256:384], qn[be][:, 0:128], ident))
            X.do(lambda be=be: nc.tensor.transpose(trp[0][:, 384:512], qn[be][:, 128:256], ident),
                 "pe", f"qtT{b}")
            X.w(f"qtcopy{b}")
            X.w(f"qpj{b-1}")  # qp_ps region free
            X.do(lambda be=be: nc.tensor.matmul(qp_ps[0], ws[:, 2:3], qt[be][:, 0:128], start=True, stop=True))
            X.do(lambda be=be: nc.tensor.matmul(qp_ps[1], ws[:, 3:4], qt[be][:, 128:256], start=True, stop=True),
                 "pe", f"qp{b}")
            # prologue T-pair for this batch's chunk 0
            k0 = b * NT
            X.w(f"cna{b}")
            X.w(f"exp_{k0-3}")  # sful[k0%3] free
            X.do(lambda k0=k0, be=be: nc.tensor.transpose(sful[k0 % 3][:, 128:256], cna[be][:, 0, 0:128], ident))
            X.do(lambda k0=k0, be=be: nc.tensor.transpose(sful[k0 % 3][:, 256:384], cna[be][:, 0, 128:256], ident),
                 "pe", f"Tpair_{k0}")
            X.w(f"qwt{b}")
            X.w(f"qpj{b}")
            for h in range(NT):
                k = b * NT + h
                sf = sful[k % 3]
                kn = k + 1
                if kn < (b + 1) * NT:
                    X.w(f"exp_{kn-3}")  # sful[kn%3] free
                    X.do(lambda kn=kn, be=be: nc.tensor.transpose(
                        sful[kn % 3][:, 128:256], cna[be][:, kn % NT, 0:128], ident))
                    X.do(lambda kn=kn, be=be: nc.tensor.transpose(
                        sful[kn % 3][:, 256:384], cna[be][:, kn % NT, 128:256], ident),
                        "pe", f"Tpair_{kn}")
                km = k - 1
                if km >= b * NT:
                    X.w(f"exp_{km}")
                    X.do(lambda km=km: nc.tensor.transpose(
                        trp[km % 2][:, 0:128], p_t[km % 4], ident), "pe", f"PT_{km}")
                kc = k - 2
                if kc >= b * NT:
                    X.w(f"ptscopy_{kc}")
                    X.do(lambda kc=kc, be=be: nc.tensor.matmul(
                        c2qp[kc % 2][:, 0:256], pts[kc % 4], qn[be], start=True, stop=True),
                        "pe", f"c2q_{kc}")
                X.w(f"ctccopy_{k}")
                X.do(lambda k=k, be=be, sf=sf: nc.tensor.matmul(
                    sf[:, 0:128], ctc[k % 4][:, 0:128], qwt[be][:, 0:128], start=True, stop=False))
                X.do(lambda k=k, be=be, sf=sf: nc.tensor.matmul(
                    sf[:, 0:128], ctc[k % 4][:, 128:256], qwt[be][:, 128:256], start=False, stop=False))
                X.do(lambda k=k, be=be, sf=sf: nc.tensor.matmul(
                    sf[:, 0:128], ones_row, qpj[be], start=False, stop=True), "pe", f"S_{k}")
                if h == 0:
                    X.w(f"x1v_{b-1}")  # cp8 region free
                X.do(lambda k=k, h=h: nc.tensor.matmul(
                    cp8[:, h : h + 1], ctc[k % 4][:, 0:128], ws[:, 0:1], start=True, stop=False))
                X.do(lambda k=k, h=h: nc.tensor.matmul(
                    cp8[:, h : h + 1], ctc[k % 4][:, 128:256], ws[:, 1:2], start=False, stop=True),
                    "pe", f"cp_{k}")
            # batch tail: PT(last), c2q(last-1), c2q(last)
            kl = b * NT + NT - 1
            X.w(f"exp_{kl}")
            X.do(lambda kl=kl: nc.tensor.transpose(trp[kl % 2][:, 0:128], p_t[kl % 4], ident),
                 "pe", f"PT_{kl}")
            for kc in (kl - 1, kl):
                X.w(f"ptscopy_{kc}")
                X.do(lambda kc=kc, be=be: nc.tensor.matmul(
                    c2qp[kc % 2][:, 0:256], pts[kc % 4], qn[be], start=True, stop=True),
                    "pe", f"c2q_{kc}")
            X.mark(f"qn_free{b}", "pe")
            # value path
            X.w(f"e8_{b}")
            X.do(lambda be=be: nc.tensor.matmul(sums8_ps, e8[be], ones_col, start=True, stop=True),
                 "pe", f"sums8mm{b}")
            X.w(f"sums8c{b}")
            X.do(lambda be=be: nc.tensor.matmul(tot_ps, sums8[be], ones_col[0:NT, :], start=True, stop=True),
                 "pe", f"totmm{b}")
            for half in range(2):
                for h in range(NT):
                    last = half == 1 and h == NT - 1
                    X.do(lambda h=h, be=be, half=half: nc.tensor.matmul(
                        q2cT[half], cna[be][:, h, 128 * half : 128 * (half + 1)],
                        e8[be][:, h : h + 1],
                        start=(h == 0), stop=(h == NT - 1)),
                        "pe" if last else None, f"q2cTmm{b}" if last else None)
            X.mark(f"cna_free{b}", "pe")
            X.w(f"q2cTc{b}")  # ACT copied q2cT to SBUF
            X.do(lambda be=be: nc.tensor.transpose(q2c_row[:, 0:128], q2cTs[be][:, 0:1], ident))
            X.do(lambda be=be: nc.tensor.transpose(q2c_row[:, 128:256], q2cTs[be][:, 1:2], ident),
                 "pe", f"q2cTT{b}")
            X.w(f"q2csb{b}")
            X.do(lambda b=b, be=be: nc.tensor.matmul(
                c2qp[b % 2][:, 256:512], ones_row, q2c_sb[be], start=True, stop=True),
                "pe", f"q2cbmm{b}")

    def stream_act(X):
        X.w("ws")
        for b in range(BL):
            be = b % 2
            X.w(f"qtT{b}")
            X.w(f"qp{b-1}")  # qt[be] free
            X.do(lambda be=be: nc.scalar.copy(qt[be], trp[0][:, 256:512]), "act", f"qtcopy{b}")
            X.w(f"qtcopy{b}")
            X.do(lambda be=be: nc.scalar.mul(qwt[be][:, 0:128], qt[be][:, 0:128], ws[:, 4:5]))
            X.do(lambda be=be: nc.scalar.mul(qwt[be][:, 128:256], qt[be][:, 128:256], ws[:, 5:6]),
                 "act", f"qwt{b}")
            k0 = b * NT
            X.w(f"Tpair_{k0}")
            X.w(f"cp_{k0-4}")
            X.do(lambda k0=k0: nc.scalar.copy(ctc[k0 % 4], sful[k0 % 3][:, 128:384]),
                 "act", f"ctccopy_{k0}")
            for h in range(NT):
                k = b * NT + h
                kn = k + 1
                if kn < (b + 1) * NT:
                    X.w(f"Tpair_{kn}")
                    X.w(f"cp_{kn-4}")  # ctc[kn%4] free
                    X.do(lambda kn=kn: nc.scalar.copy(ctc[kn % 4], sful[kn % 3][:, 128:384]),
                         "act", f"ctccopy_{kn}")
                X.w(f"S_{k}")
                X.w(f"PT_{k-4}")  # p_t[k%4] free (PE reader)
                X.w(f"ssum_{k-4}")  # p_t[k%4] free (DVE reader)
                X.do(lambda k=k: nc.scalar.activation(p_t[k % 4], sful[k % 3][:, 0:128], EXP),
                     "act", f"exp_{k}")
                km = k - 1
                if km >= b * NT:
                    X.w(f"PT_{km}")
                    X.w(f"c2q_{km-4}")  # pts[km%4] free
                    X.do(lambda km=km: nc.scalar.copy(pts[km % 4], trp[km % 2][:, 0:128]),
                         "act", f"ptscopy_{km}")
                kc = k - 2
                if kc >= b * NT:
                    X.w(f"c2q_{kc}")
                    X.w(f"rs01_{kc}")
                    X.w(f"gfree_{kc - NG}")
                    X.do(lambda kc=kc: nc.scalar.mul(
                        g[kc % NG][:, 256:512], c2qp[kc % 2][:, 0:256], rs01[kc % 4]),
                        "act", f"G1_{kc}")
            kl = b * NT + NT - 1
            X.w(f"PT_{kl}")
            X.do(lambda kl=kl: nc.scalar.copy(pts[kl % 4], trp[kl % 2][:, 0:128]),
                 "act", f"ptscopy_{kl}")
            for kc in (kl - 1, kl):
                X.w(f"c2q_{kc}")
                X.w(f"rs01_{kc}")
                X.w(f"gfree_{kc - NG}")
                X.do(lambda kc=kc: nc.scalar.mul(
                    g[kc % NG][:, 256:512], c2qp[kc % 2][:, 0:256], rs01[kc % 4]),
                    "act", f"G1_{kc}")
            X.w(f"x1v_{b}")
            X.do(lambda be=be: nc.scalar.activation(ex8[be], x1[be], EXP), "act", f"ex8_{b}")
            X.w(f"q2cTmm{b}")
            X.do(lambda be=be: nc.scalar.copy(q2cTs[be], auxp[:, 8:10]), "act", f"q2cTc{b}")
            X.w(f"q2cbmm{b}")
            X.do(lambda b=b, be=be: nc.scalar.copy(q2cb[be], c2qp[b % 2][:, 256:512]),
                 "act", f"q2cbcopy{b}")

    def stream_dve(X):
        for b in range(BL):
            be = b % 2
            X.w(f"qp{b}")
            X.w(f"masks{b}")
            X.do(lambda be=be: nc.vector.tensor_copy(qpj[be], qp_ps[0]), "dve", f"qpj0{b}")
            X.w(f"qpj0{b}")
            X.do(lambda be=be: nc.vector.tensor_add(qpj[be], qpj[be], qp_ps[1]),
                 "dve", f"qpjh{b}")
            X.w(f"qpjh{b}")
            X.do(lambda be=be: nc.vector.tensor_add(qpj[be], qpj[be], jmq[be]), "dve", f"qpj{b}")
            X.w(f"cna{b}")
            k0 = b * NT

            def dve_rcp(kk):
                X.w(f"ssum_{kk}")
                X.do(lambda kk=kk: nc.vector.reciprocal(rcp[kk % 4], ssum[kk % 4]),
                     "dve", f"rcp_{kk}")

            def dve_rs01(kk, bb):
                X.w(f"rcp_{kk}")
                X.do(lambda kk=kk, bb=bb: nc.vector.tensor_mul(
                    rs01[kk % 4], t018[bb % 2][:, (kk % NT) : (kk % NT) + 1], rcp[kk % 4]),
                    "dve", f"rs01_{kk}")

            def dve_g2(kk):
                X.w(f"G1_{kk}")
                X.w(f"G0_{kk}")
                X.do(lambda kk=kk: nc.vector.tensor_mul(
                    g[kk % NG][:, 512:768], g[kk % NG][:, 0:256], g[kk % NG][:, 256:512]),
                    "dve", f"G2_{kk}")

            for h in range(NT):
                k = k0 + h
                X.w(f"exp_{k}")
                X.do(lambda k=k, h=h, be=be: nc.vector.reduce_max(
                    pm8[be][:, h : h + 1], p_t[k % 4], axis=AX))
                X.do(lambda k=k: nc.vector.reduce_sum(ssum[k % 4], p_t[k % 4], axis=AX),
                     "dve", f"ssum_{k}")
                if k - 1 >= k0:
                    dve_rcp(k - 1)
                if k - 2 >= k0:
                    dve_rs01(k - 2, b)
            kl = k0 + NT - 1
            dve_rcp(kl)
            dve_rs01(kl - 1, b)
            dve_rs01(kl, b)

            # value path
            X.w(f"cp_{kl}")
            X.do(lambda be=be: nc.vector.tensor_tensor(x1[be], cp8, tm8[be], op=ADD),
                 "dve", f"x1_{b}")
            X.mark(f"x1v_{b}", "dve")
            X.mark(f"masks_free{b}", "dve")
            X.w(f"ex8_{b}")
            X.w(f"ssum_{kl}")  # pm8 writes complete
            X.do(lambda be=be: nc.vector.tensor_mul(e8[be], pm8[be], ex8[be]), "dve", f"e8_{b}")
            X.w(f"sums8mm{b}")
            X.do(lambda be=be: nc.vector.tensor_copy(sums8[be], sums8_ps), "dve", f"sums8c{b}")
            X.w(f"totmm{b}")
            X.do(lambda be=be: nc.vector.reciprocal(rtot[be], tot_ps), "dve", f"rtot{b}")
            X.w(f"q2cTT{b}")
            X.w(f"rtot{b}")
            X.do(lambda be=be: nc.vector.tensor_scalar_mul(q2c_sb[be], q2c_row, rtot[be]),
                 "dve", f"q2csb{b}")

            X.mark(f"qn_free{b}_unused", "dve")

    streams = [("sync", stream_sync), ("gpsimd", stream_pool), ("tensor", stream_pe),
               ("scalar", stream_act), ("vector", stream_dve)]

    # pass 1: dry run to collect events
    ev = {}
    ctr = {n: 0 for n in sem_names}
    for _, s in streams:
        s(Em(True, ctr, ev, None, None))
    dry_ctr = dict(ctr)

    # pass 2: real emission
    ctr2 = {n: 0 for n in sem_names}
    with nc.Block() as block:

        @block.sync
        def _(eng):
            stream_sync(Em(False, ctr2, ev, eng, sems, own=None))

        @block.gpsimd
        def _(eng):
            stream_pool(Em(False, ctr2, ev, eng, sems, own="pool"))

        @block.tensor
        def _(eng):
            stream_pe(Em(False, ctr2, ev, eng, sems, own="pe"))

        @block.scalar
        def _(eng):
            stream_act(Em(False, ctr2, ev, eng, sems, own="act"))

        @block.vector
        def _(eng):
            stream_dve(Em(False, ctr2, ev, eng, sems, own="dve"))

    assert ctr2 == dry_ctr, (ctr2, dry_ctr)
    return nc

@functools.lru_cache(maxsize=1)
def _build_cached():
    return build()


def _host_prep(context, question, con_lens, qu_lens, att_w):
    context = np.asarray(context, dtype=np.float32)
    question = np.ascontiguousarray(np.asarray(question, dtype=np.float32))
    con = np.asarray(con_lens).astype(np.int64)
    qu = np.asarray(qu_lens).astype(np.int64)
    w = np.asarray(att_w, dtype=np.float32).reshape(3, D)

    t01 = (np.arange(T)[None, :] < con[:, None]).astype(np.float32)  # (B, T)
    # pre-zero invalid context rows (see module docstring)
    context = np.ascontiguousarray(context * t01.T[:, :, None])
    # [b, p, c] = t01[b, c*128 + p]
    t01t = np.ascontiguousarray(t01.reshape(B, NT, 128).transpose(0, 2, 1))
    tmt = np.ascontiguousarray(((1.0 - t01t) * NEG).astype(np.float32))
    jmq = np.where(np.arange(J)[None, :] < qu[:, None], 0.0, NEG).astype(np.float32)
    jmq = np.ascontiguousarray(jmq[:, None, :])  # (B, 1, J)
    wsT = np.ascontiguousarray(
        np.stack(
            [w[0, :128], w[0, 128:], w[1, :128], w[1, 128:], w[2, :128], w[2, 128:]],
            axis=1,
        )
    )  # (128, 6)
    return context, question, t01t, tmt, jmq, wsT


def kernel(context, question, con_lens, qu_lens, att_w):
    from concourse.bass_utils import run_bass_kernel_spmd

    context, question, t01t, tmt, jmq, wsT = _host_prep(
        context, question, con_lens, qu_lens, att_w
    )
    in_maps = []
    for i in range(NCORES):
        sl = slice(i * BL, (i + 1) * BL)
        in_maps.append(
            {
                "context": np.ascontiguousarray(context[:, sl, :]),
                "question": np.ascontiguousarray(question[:, sl, :]),
                "wsT": wsT,
                "t01t": t01t[sl],
                "tmaskt": tmt[sl],
                "jmq": jmq[sl],
            }
        )
    nc = _build_cached()
    res = run_bass_kernel_spmd(nc, in_maps, core_ids=list(range(NCORES)))
    out = np.concatenate(
        [np.asarray(res.results[i]["out"]).reshape(BL, T, 4 * D) for i in range(NCORES)],
        axis=0,
    )
    return out



# revision 29
# speedup vs baseline: 1.6585x; 1.6585x over previous
"""Trainium2 Bass kernel (raw Bass, explicit semaphores) for a BiDAF-style
attention-flow layer — bf16 restructure.

Math (per batch b):
    S[t,j] = c.w_c + q.w_q + (c*q).w_cq, masked by (t<con_len)&(j<qu_len)
    c2q    = softmax_j(S) @ Q
    value  = softmax_t(max_j S);  q2c = sum_t value[t] * C[t]
    G      = [C, c2q, C*c2q, C*q2c] * t_valid

Device design (vs the fp32 t-major baseline):
  - All matmuls in bf16 (1 cyc/row vs fp32's 4); PSUM accumulates fp32.
  - S is computed TRANSPOSED (S'[j,t]) so q_proj[j] + jmask[j] folds into
    the per-partition bias of the exp activation, and exp(S') IS the
    lhsT P^T needed by the c2q matmul — no PT transposes / pts copies.
  - Value path: P^T is transposed back per chunk on PE (bf16) and DVE
    reduce_max/reduce_sum over free-j give pm8 and the softmax sums.
  - NO narrow (N=1) bf16 matmuls: they corrupt PSUM/crash on real HW at
    pipeline rate (found by bisection; fp32 baseline was immune).
    * c_proj+tmask (x1) and q_proj+jmask (exp bias) are host-precomputed
      mask-style aux columns (tiny linear input projections).
    * q2c^T uses N=8-wide accumulating matmuls against an 8x8
      identity-masked e8 (e8m), summed with one DVE reduce.
    * value-sum total via gpsimd XYZWC reduce; q2c broadcast via gpsimd
      partition_broadcast (no K=1 rank-1 matmul).
  - G0 (= masked context, a verbatim input copy) is assembled on host;
    device emits only [c2q, C*c2q, C*q2c] as bf16 (host upcasts), which
    cuts store DMA from 16 MB to 6 MB per core.
  - One coarse DMA per tensor per batch (HWDGE is a serialized ~625ns
    per-DMA resource); contiguous runs kept >= 512B for full bandwidth.
  - Two-pass emission: dry pass records semaphore values, real pass
    emits standalone wait_ge commands.
Sharding: data-parallel over batch B=32 across 8 NeuronCores (4 each).
"""

import os
import sys
import functools

BISECT = int(os.environ.get("KBISECT", "0"))   # 1 = drop value path (debug)

for _p in ("/opt/trn_rl_repo",):
    if _p not in sys.path:
        sys.path.insert(0, _p)

import numpy as np
import ml_dtypes
import concourse.bass as bass
from concourse import mybir

T, J, B, D = 1024, 128, 32, 256
NCORES = 8
BL = B // NCORES
NT = T // 128
NEG = -1.0e30

F32 = mybir.dt.float32
BF16 = mybir.dt.bfloat16
AX = mybir.AxisListType.X
AXC = mybir.AxisListType.XYZWC
EXP = mybir.ActivationFunctionType.Exp
ADD = mybir.AluOpType.add

DMA_SEMS = {"wsb", "c0", "c1", "x0", "x1", "q0", "q1", "t0", "t1", "m0", "m1",
            "g0", "g1"}


class Em:
    """Per-engine emitter: dry pass counts sem values, real pass emits."""

    def __init__(self, dry, ctr, ev, eng=None, sems=None):
        self.dry = dry
        self.ctr = ctr
        self.ev = ev
        self.eng = eng
        self.sems = sems
        self.waited = {}

    def do(self, fn, sem=None, tag=None):
        inst = None if self.dry else fn()
        if sem is not None:
            step = 16 if sem in DMA_SEMS else 1
            if inst is not None:
                inst.then_inc(self.sems[sem], step)
            self.ctr[sem] += step
            if tag is not None:
                self.ev[tag] = (sem, self.ctr[sem])
        return inst

    def mark(self, tag, sem):
        self.ev[tag] = (sem, self.ctr[sem])

    def w(self, tag):
        if self.dry:
            return
        if tag not in self.ev:
            return
        sem, val = self.ev[tag]
        if val <= 0:
            return
        if self.waited.get(sem, 0) >= val:
            return
        self.eng.wait_ge(self.sems[sem], val)
        self.waited[sem] = val


def build():
    nc = bass.Bass("TRN2", target_bir_lowering=False, debug=False)

    cna_d = nc.dram_tensor("cnab", (BL, 128, NT * D), BF16, kind="ExternalInput").ap()
    ctxT_d = nc.dram_tensor("ctxTb", (BL, 128, 2 * NT * 128), BF16, kind="ExternalInput").ap()
    qn_d = nc.dram_tensor("qnb", (BL, 128, D), BF16, kind="ExternalInput").ap()
    qT_d = nc.dram_tensor("qTb", (BL, 128, 2 * J), BF16, kind="ExternalInput").ap()
    aux_d = nc.dram_tensor("auxf", (BL, 128, 20), F32, kind="ExternalInput").ap()
    wsb_d = nc.dram_tensor("wsb", (128, 68), BF16, kind="ExternalInput").ap()
    out_d = nc.dram_tensor("out", (BL, T, 3 * D), BF16, kind="ExternalOutput").ap()

    A = lambda name, shape, dt=BF16: nc.alloc_sbuf_tensor(name, list(shape), dt).ap()

    identb = A("identb", (128, 128))
    ones_row = A("ones_row", (1, 128))
    wsb = A("wsb_s", (128, 68))
    i8v = wsb[:, 4:68].rearrange("p (h k) -> p h k", k=8)
    cna = [A(f"cna{i}", (128, NT, D)) for i in range(2)]
    ctxT = [A(f"ctxT{i}", (128, 2, NT, 128)) for i in range(2)]
    qn = [A(f"qn{i}", (128, D)) for i in range(2)]
    qT = [A(f"qT{i}", (128, 2 * J)) for i in range(2)]
    aux = [A(f"aux{i}", (128, 20), F32) for i in range(2)]
    qwT = [A(f"qwT{i}", (128, 2 * J)) for i in range(2)]
    PT = [A(f"PT{i}", (128, NT, 128)) for i in range(2)]
    pm8 = [A(f"pm8_{i}", (128, NT), F32) for i in range(2)]
    ss8 = [A(f"ss8_{i}", (128, NT), F32) for i in range(2)]
    rcp8 = [A(f"rcp8_{i}", (128, NT), F32) for i in range(2)]
    rs018 = [A(f"rs018_{i}", (128, NT), F32) for i in range(2)]
    ex8 = [A(f"ex8_{i}", (128, NT), F32) for i in range(2)]
    e8 = [A(f"e8_{i}", (128, NT)) for i in range(2)]
    e8m = [A(f"e8m_{i}", (128, NT, NT)) for i in range(2)]
    totsb = [A(f"totsb_{i}", (1, 1), F32) for i in range(2)]
    rtot = [A(f"rtot_{i}", (1, 1), F32) for i in range(2)]
    q2cTf = [A(f"q2cTf{i}", (128, 2), F32) for i in range(2)]
    q2cTs = [A(f"q2cTs{i}", (128, 2)) for i in range(2)]
    q2c_sb = [A(f"q2c_sb{i}", (1, D)) for i in range(2)]
    q2cb = [A(f"q2cb{i}", (128, D)) for i in range(2)]
    gbig = [A(f"gbig{i}", (128, NT, 3, D)) for i in range(2)]

    P = lambda name, shape, dt=F32: nc.alloc_psum_tensor(name, list(shape), dt).ap()
    sp = [P(f"sp{i}", (128, 512)) for i in range(2)]        # S' lo/hi (4 chunks each)
    c2q_ps = P("c2q_ps", (128, NT * D))                      # 4 banks
    pback = P("pback", (128, NT * 128), BF16)                # 1 bank, bf16
    auxp = P("auxp", (128, 512))                             # 1 bank
    q2cT8 = [auxp[:, 0:8], auxp[:, 8:16]]                    # (128, 8) each
    q2cT8v = auxp[:, 0:16].rearrange("p (a k) -> p a k", k=8)
    q2c_rowb = auxp[0:1, 32:160].bitcast(BF16)               # (1, 256) bf16
    q2cb_ps = auxp[:, 256:512]

    sem_names = list(DMA_SEMS) + ["pe", "act", "dve", "pool"]
    sems = {n: nc.alloc_semaphore(f"sem_{n}") for n in sem_names}

    pbv = pback.rearrange("p (c j) -> p c j", j=128)

    # ------------------------------------------------------------------ streams
    def stream_sync(X):
        X.do(lambda: nc.sync.dma_start(out=wsb, in_=wsb_d), "wsb", "wsb")
        for b in range(BL):
            be = b % 2
            sfx = str(be)
            # loads for b (buffer-free waits are on batch b-2 consumers)
            X.w(f"q2cTmm_{b-2}"); X.w(f"G2_{b-2}"); X.w(f"G3_{b-2}")
            X.do(lambda b=b, be=be: nc.sync.dma_start(
                out=cna[be].rearrange("p c d -> p (c d)"), in_=cna_d[b]),
                f"c{sfx}", f"c_{b}")
            X.w(f"sh_{b-2}")
            X.do(lambda b=b, be=be: nc.sync.dma_start(
                out=ctxT[be].rearrange("p a c t -> p (a c t)"), in_=ctxT_d[b]),
                f"x{sfx}", f"x_{b}")
            X.w(f"c2qh_{b-2}")
            X.do(lambda b=b, be=be: nc.sync.dma_start(out=qn[be], in_=qn_d[b]),
                 f"q{sfx}", f"q_{b}")
            X.w(f"qwT_{b-2}")
            X.do(lambda b=b, be=be: nc.sync.dma_start(out=qT[be], in_=qT_d[b]),
                 f"t{sfx}", f"t_{b}")
            X.w(f"rs018_{b-2}"); X.w(f"ex8_{b-2}"); X.w(f"exph_{b-2}")
            X.do(lambda b=b, be=be: nc.sync.dma_start(out=aux[be], in_=aux_d[b]),
                 f"m{sfx}", f"m_{b}")
            if b >= 1:
                store(X, b - 1)
        store(X, BL - 1)

    def store(X, b):
        be = b % 2
        X.w(f"G2_{b}"); X.w(f"G3_{b}"); X.w(f"G1a_{b}"); X.w(f"G1d_{b}")
        X.do(lambda b=b, be=be: nc.sync.dma_start(
            out=out_d[b].rearrange("(c p) d -> p c d", p=128),
            in_=gbig[be].rearrange("p c three d -> p c (three d)")),
            f"g{be}", f"gfree_{b}")

    def stream_pool(X):
        NE = mybir.AluOpType.not_equal
        X.do(lambda: nc.gpsimd.memset(identb, 0.0), "pool", "identms")
        if not X.dry:
            X.eng.wait_ge(sems["pool"], X.ev["identms"][1])
        X.do(lambda: nc.gpsimd.affine_select(
            out=identb, in_=identb, compare_op=NE, fill=1.0, base=0,
            pattern=[[-1, 128]], channel_multiplier=1), "pool")
        X.do(lambda: nc.gpsimd.memset(ones_row, 1.0), "pool", "consts")
        if BISECT == 1:
            return
        for b in range(BL):
            be = b % 2
            # value total: tot = sum_{t,h} e8  (gpsimd full reduce)
            X.w(f"e8_{b}")
            X.w(f"rtot_{b-2}")  # totsb[be] free (DVE reader)
            X.do(lambda be=be: nc.gpsimd.tensor_reduce(
                totsb[be], e8[be], axis=AXC, op=ADD), "pool", f"totred_{b}")
            # G3 = cna * q2cb
            X.w(f"q2cbc_{b}")
            X.w(f"c_{b}")
            X.w(f"gfree_{b-2}")
            X.do(lambda be=be: nc.gpsimd.tensor_mul(
                gbig[be][:, :, 2, :], cna[be],
                q2cb[be].unsqueeze(1).broadcast_to((128, NT, D))),
                "pool", f"G3_{b}")

    def stream_pe(X):
        X.w("consts")
        X.w("wsb")
        for b in range(BL):
            be = b % 2
            # A/B: S' halves (4 chunks each)
            X.w(f"x_{b}")
            X.w(f"qwT_{b}")
            for half in range(2):
                X.w(f"exp{'lh'[half]}_{b-1}")  # sp[half] free
                for hh in range(4):
                    h = half * 4 + hh
                    X.do(lambda be=be, half=half, h=h, hh=hh: nc.tensor.matmul(
                        sp[half][:, hh * 128:(hh + 1) * 128],
                        qwT[be][:, 0:128], ctxT[be][:, 0, h, :],
                        start=True, stop=False))
                    X.do(lambda be=be, half=half, h=h, hh=hh: nc.tensor.matmul(
                        sp[half][:, hh * 128:(hh + 1) * 128],
                        qwT[be][:, 128:256], ctxT[be][:, 1, h, :],
                        start=False, stop=True),
                        "pe", f"s{'lh'[half]}_{b}" if hh == 3 else None)
            # C/D: c2q + pback per half
            X.w(f"q_{b}")
            for half in range(2):
                X.w(f"exp{'lh'[half]}_{b}")  # PT half ready
                if half == 0:
                    X.w(f"G1a_{b-1}")  # c2q_ps lo free
                    X.w(f"maxred_{b-1}"); X.w(f"sumred_{b-1}")  # pback free
                else:
                    X.w(f"G1d_{b-1}")  # c2q_ps hi free
                for hh in range(4):
                    h = half * 4 + hh
                    X.do(lambda be=be, h=h: nc.tensor.matmul(
                        c2q_ps[:, h * D:(h + 1) * D], PT[be][:, h, :], qn[be],
                        start=True, stop=True),
                        "pe", f"c2q{'lh'[half]}_{b}" if hh == 3 else None)
                    X.do(lambda be=be, h=h: nc.tensor.transpose(
                        pback[:, h * 128:(h + 1) * 128], PT[be][:, h, :], identb),
                        "pe", f"pb{'lh'[half]}_{b}" if hh == 3 else None)
            X.mark(f"ptfree_{b}", "pe")
            if BISECT == 1:
                continue
            # E: q2c^T via N=8 identity-masked accumulating matmuls
            X.w(f"e8m_{b}")
            X.w(f"q2cTf_{b-1}")  # q2cT8 psum free (DVE reader)
            for half in range(2):
                for h in range(NT):
                    X.do(lambda be=be, h=h, half=half: nc.tensor.matmul(
                        q2cT8[half], cna[be][:, h, 128 * half:128 * (half + 1)],
                        e8m[be][:, h, :],
                        start=(h == 0), stop=(h == NT - 1)),
                        "pe" if (h == NT - 1 and half == 1) else None,
                        f"q2cTmm_{b}" if (h == NT - 1 and half == 1) else None)
            # F: q2c row transposes
            X.w(f"q2cTs_{b}")
            X.w(f"q2csb_{b-1}")  # q2c_rowb free
            X.do(lambda be=be: nc.tensor.transpose(
                q2c_rowb[:, 0:128], q2cTs[be][:, 0:1], identb))
            X.do(lambda be=be: nc.tensor.transpose(
                q2c_rowb[:, 128:256], q2cTs[be][:, 1:2], identb),
                "pe", f"q2cTT_{b}")
            # G: q2cb rank-1 broadcast (K=1, N=256 — wide write)
            X.w(f"q2csb_{b}")
            X.w(f"q2cbc_{b-1}")  # q2cb_ps free
            X.do(lambda be=be: nc.tensor.matmul(
                q2cb_ps, ones_row, q2c_sb[be], start=True, stop=True),
                "pe", f"q2cbmm_{b}")

    def stream_act(X):
        for b in range(BL):
            be = b % 2
            # qwT for this batch
            X.w(f"t_{b}"); X.w(f"m_{b}")
            X.w(f"sh_{b-2}")  # qwT[be] free
            X.do(lambda be=be: nc.scalar.mul(
                qwT[be][:, 0:128], qT[be][:, 0:128], aux[be][:, 17:18]))
            X.do(lambda be=be: nc.scalar.mul(
                qwT[be][:, 128:256], qT[be][:, 128:256], aux[be][:, 18:19]),
                "act", f"qwT_{b}")
            # value path exp (x1 = c_proj + tmask is host-precomputed)
            if BISECT != 1:
                X.w(f"e8_{b-2}")  # ex8[be] free (DVE reader)
                X.do(lambda be=be: nc.scalar.activation(
                    ex8[be], aux[be][:, 8:16], EXP), "act", f"ex8_{b}")
            # exp halves (bias = q_proj + jmask, host-precomputed)
            for half in range(2):
                X.w(f"s{'lh'[half]}_{b}")
                if half == 0:
                    X.w(f"ptfree_{b-2}")  # PT[be] free
                X.do(lambda be=be, half=half: nc.scalar.activation(
                    PT[be][:, half * 4:half * 4 + 4, :], sp[half], EXP,
                    bias=aux[be][:, 16:17]), "act", f"exp{'lh'[half]}_{b}")
            # G1 chunks 0-3 (per-chunk: scale by rs01 column)
            X.w(f"c2ql_{b}")
            X.w(f"rs018_{b}")
            X.w(f"gfree_{b-2}")
            for h in range(4):
                X.do(lambda be=be, h=h: nc.scalar.mul(
                    gbig[be][:, h, 0, :], c2q_ps[:, h * D:(h + 1) * D],
                    rs018[be][:, h:h + 1]),
                    "act", f"G1a_{b}" if h == 3 else None)
            # q2cTs: bf16 copy of the DVE-reduced q2c^T halves
            if BISECT != 1:
                X.w(f"q2cTf_{b}")
                X.w(f"q2cTT_{b-2}")  # q2cTs[be] free (PE reader)
                X.do(lambda be=be: nc.scalar.copy(q2cTs[be], q2cTf[be]),
                     "act", f"q2cTs_{b}")
                X.w(f"q2cbmm_{b}")
                X.w(f"G3_{b-2}")  # q2cb[be] free (pool reader)
                X.do(lambda be=be: nc.scalar.copy(q2cb[be], q2cb_ps),
                     "act", f"q2cbc_{b}")

    def stream_dve(X):
        for b in range(BL):
            be = b % 2
            X.w(f"pbh_{b}")
            X.do(lambda be=be: nc.vector.reduce_max(pm8[be], pbv, axis=AX),
                 "dve", f"maxred_{b}")
            X.do(lambda be=be: nc.vector.reduce_sum(ss8[be], pbv, axis=AX),
                 "dve", f"sumred_{b}")
            X.w(f"sumred_{b}")
            X.do(lambda be=be: nc.vector.reciprocal(rcp8[be], ss8[be]), "dve",
                 f"rcp_{b}")
            X.w(f"rcp_{b}"); X.w(f"m_{b}")
            X.w(f"G1a_{b-2}")  # rs018[be] free (ACT reader)
            X.do(lambda be=be: nc.vector.tensor_mul(
                rs018[be], rcp8[be], aux[be][:, 0:8]), "dve", f"rs018_{b}")
            if BISECT != 1:
                X.w(f"ex8_{b}")
                X.w(f"maxred_{b}")
                X.w(f"totred_{b-2}")  # e8[be] free (pool reader)
                X.do(lambda be=be: nc.vector.tensor_mul(e8[be], pm8[be], ex8[be]),
                     "dve", f"e8_{b}")
                # e8m[t, h, k] = e8[t, h] * (h == k)
                X.w(f"e8_{b}")
                X.w(f"q2cTmm_{b-2}")  # e8m[be] free (PE reader)
                X.do(lambda be=be: nc.vector.tensor_mul(
                    e8m[be], e8[be].unsqueeze(2).broadcast_to((128, NT, NT)),
                    i8v), "dve", f"e8m_{b}")
                # q2cTf = sum_h of the masked matmul columns
                X.w(f"q2cTmm_{b}")
                X.w(f"q2cTs_{b-2}")  # q2cTf[be] free (ACT reader)
                X.do(lambda be=be: nc.vector.reduce_sum(
                    q2cTf[be], q2cT8v, axis=AX), "dve", f"q2cTf_{b}")
                # value normalization
                X.w(f"totred_{b}")
                X.do(lambda be=be: nc.vector.reciprocal(rtot[be], totsb[be]),
                     "dve", f"rtot_{b}")
                X.w(f"q2cTT_{b}")
                X.w(f"rtot_{b}")
                X.w(f"q2cbmm_{b-2}")  # q2c_sb[be] free (PE reader)
                X.do(lambda be=be: nc.vector.tensor_scalar_mul(
                    q2c_sb[be], q2c_rowb, rtot[be]), "dve", f"q2csb_{b}")
            # G1 chunks 4-7 (per-chunk, scale col broadcast over d)
            X.w(f"c2qh_{b}")
            X.w(f"rs018_{b}")
            X.w(f"gfree_{b-2}")
            for h in range(4, 8):
                X.do(lambda be=be, h=h: nc.vector.tensor_mul(
                    gbig[be][:, h, 0, :], c2q_ps[:, h * D:(h + 1) * D],
                    rs018[be][:, h:h + 1].broadcast_to((128, D))),
                    "dve", f"G1d_{b}" if h == 7 else None)
            # G2 = cna * G1 (all-bf16, per chunk)
            X.w(f"G1a_{b}")
            X.w(f"G1d_{b}")
            X.w(f"c_{b}")
            for h in range(NT):
                X.do(lambda be=be, h=h: nc.vector.tensor_mul(
                    gbig[be][:, h, 1, :], cna[be][:, h, :], gbig[be][:, h, 0, :]),
                    "dve", f"G2_{b}" if h == NT - 1 else None)
            if BISECT == 1:
                for h in range(NT):
                    X.do(lambda be=be, h=h: nc.vector.tensor_copy(
                        gbig[be][:, h, 2, :], cna[be][:, h, :]),
                        "dve", f"G3_{b}" if h == NT - 1 else None)

    streams = [("sync", stream_sync), ("gpsimd", stream_pool),
               ("tensor", stream_pe), ("scalar", stream_act),
               ("vector", stream_dve)]

    # pass 1: dry run to collect events
    ev = {}
    ctr = {n: 0 for n in sem_names}
    for _, s in streams:
        s(Em(True, ctr, ev, None, None))
    dry_ctr = dict(ctr)

    # pass 2: real emission
    ctr2 = {n: 0 for n in sem_names}
    with nc.Block() as block:

        @block.sync
        def _(eng):
            stream_sync(Em(False, ctr2, ev, eng, sems))

        @block.gpsimd
        def _(eng):
            stream_pool(Em(False, ctr2, ev, eng, sems))

        @block.tensor
        def _(eng):
            stream_pe(Em(False, ctr2, ev, eng, sems))

        @block.scalar
        def _(eng):
            stream_act(Em(False, ctr2, ev, eng, sems))

        @block.vector
        def _(eng):
            stream_dve(Em(False, ctr2, ev, eng, sems))

    assert ctr2 == dry_ctr, (ctr2, dry_ctr)
    return nc


@functools.lru_cache(maxsize=1)
def _build_cached():
    return build()


def _host_prep(context, question, con_lens, qu_lens, att_w):
    bf = ml_dtypes.bfloat16
    ctx = np.asarray(context, dtype=np.float32)      # (T, B, D)
    q = np.asarray(question, dtype=np.float32)       # (J, B, D)
    con = np.asarray(con_lens).astype(np.int64)
    qu = np.asarray(qu_lens).astype(np.int64)
    w = np.asarray(att_w, dtype=np.float32).reshape(3, D)

    t01 = (np.arange(T)[None, :] < con[:, None]).astype(np.float32)   # (B, T)
    ctxz = np.ascontiguousarray(ctx * t01.T[:, :, None])              # zeroed pads
    ctx_bt = ctxz.transpose(1, 0, 2)                                  # (B, T, D)

    cnab = np.ascontiguousarray(
        ctx_bt.reshape(B, NT, 128, D).transpose(0, 2, 1, 3)
        .reshape(B, 128, NT * D).astype(bf))
    ctxTb = np.ascontiguousarray(
        ctx_bt.transpose(0, 2, 1)                      # (B, D, T)
        .reshape(B, 2, 128, NT, 128).transpose(0, 2, 1, 3, 4)
        .reshape(B, 128, 2 * NT * 128).astype(bf))
    qnb = np.ascontiguousarray(q.transpose(1, 0, 2).astype(bf))        # (B, J, D)
    qTb = np.ascontiguousarray(
        q.transpose(1, 2, 0)                            # (B, D, J)
        .reshape(B, 2, 128, J).transpose(0, 2, 1, 3)
        .reshape(B, 128, 2 * J).astype(bf))
    t01t = t01.reshape(B, NT, 128).transpose(0, 2, 1)   # (B, 128, NT)
    # tiny host-side linear projections (narrow device matmuls are unsafe)
    c_proj = np.einsum("tbd,d->bt", ctx, w[0])          # (B, T)
    q_proj = np.einsum("jbd,d->bj", q, w[1])            # (B, J)
    cpt = c_proj.reshape(B, NT, 128).transpose(0, 2, 1)  # (B, 128, NT)
    auxf = np.zeros((B, 128, 20), dtype=np.float32)
    auxf[:, :, 0:8] = t01t
    auxf[:, :, 8:16] = cpt + (1.0 - t01t) * NEG          # x1 = c_proj + tmask
    auxf[:, :, 16] = q_proj + np.where(
        np.arange(J)[None, :] < qu[:, None], 0.0, NEG)   # exp bias
    auxf[:, :, 17] = w[2, 0:128][None, :]
    auxf[:, :, 18] = w[2, 128:256][None, :]
    wsbf = np.zeros((128, 68), dtype=np.float32)
    wsbf[:, 4:68] = np.eye(NT, dtype=np.float32).reshape(1, NT * NT)
    wsb = np.ascontiguousarray(wsbf.astype(bf))
    return cnab, ctxTb, qnb, qTb, auxf, wsb, ctx_bt


def kernel(context, question, con_lens, qu_lens, att_w):
    from concourse.bass_utils import run_bass_kernel_spmd

    cnab, ctxTb, qnb, qTb, auxf, wsb, ctx_bt = _host_prep(
        context, question, con_lens, qu_lens, att_w)
    in_maps = []
    for i in range(NCORES):
        sl = slice(i * BL, (i + 1) * BL)
        in_maps.append({
            "cnab": np.ascontiguousarray(cnab[sl]),
            "ctxTb": np.ascontiguousarray(ctxTb[sl]),
            "qnb": np.ascontiguousarray(qnb[sl]),
            "qTb": np.ascontiguousarray(qTb[sl]),
            "auxf": np.ascontiguousarray(auxf[sl]),
            "wsb": wsb,
        })
    nc = _build_cached()
    res = run_bass_kernel_spmd(nc, in_maps, core_ids=list(range(NCORES)))
    dev = np.concatenate(
        [np.asarray(res.results[i]["out"]).reshape(BL, T, 3 * D)
         for i in range(NCORES)], axis=0).astype(np.float32)   # (B, T, 768)
    out = np.empty((B, T, 4 * D), dtype=np.float32)
    out[:, :, 0:D] = ctx_bt          # G0 = masked context (verbatim input)
    out[:, :, D:] = dev              # [c2q, C*c2q, C*q2c]
    return out
